# revision 16
# baseline (speedup 1.0000x reference)
"""Trainium2 Bass kernel for nn_BatchedFasterRCNN (histogram binning + per-cell top-3).

Contract: kernel(**inputs) takes FULL inputs (boxes [256,1000,4] f32,
labels [256,1000] int, pred_scores [256,1000] f32, H=480, W=640) and returns
(classes_out [256,3,7,7] int, boxes_out [256,15,7,7] f32), matching reference.

Strategy: pure data-parallel over 8 NeuronCores (32 images each). Per core:
  A) elementwise binning at [128,250] (partition = image-quarter):
     nb = boxes * (1/480); px/py by 6 threshold compares each;
     bin = 7*py+px; smask = score * (score > 0.12); per-box feature table
     [score, w, h, x0, y0, label] staged to DRAM.
  B) 16 tiles of [98,1000] (2 images x 49 cells): replicate smask (f32) and
     bin (bf16) rows across each image's 49 cell-partitions via broadcast-AP
     SBUF->SBUF DMA; mask = (bin == cell); masked = smask * mask;
     vector.max -> top-8 scores; vector.max_index -> their box indices.
  C) indirect-DMA gather of the top-3 boxes' feature rows from the DRAM
     table; validity masking (-1 / 0 fills); DMA to transposed outputs.
"""

import os
import numpy as np

N_CORES = 8
B_FULL, N = 256, 1000
B = B_FULL // N_CORES  # 32 images per core
RES = 7
R2 = RES * RES  # 49
K = 3
THRES = 0.12
H_CONST, W_CONST = 480, 640

LAST_RESULTS = None  # BassKernelResults of the most recent run (for profiling)

_PROGRAM = None


def _build_program(split_waits=True):
    from contextlib import ExitStack

    import concourse.bass as bass
    import concourse.tile as tile
    import concourse.mybir as mybir

    f32 = mybir.dt.float32
    bf16 = mybir.dt.bfloat16
    i32 = mybir.dt.int32
    u32 = mybir.dt.uint32
    Alu = mybir.AluOpType

    nc = bass.Bass()

    scores_d = nc.dram_tensor("scores", [B, N], f32, kind="ExternalInput")
    boxes_d = nc.dram_tensor("boxes", [B, N, 4], f32, kind="ExternalInput")
    labels_d = nc.dram_tensor("labels", [B, N], f32, kind="ExternalInput")
    boxout_d = nc.dram_tensor("boxout", [B, K * 5, RES, RES], f32, kind="ExternalOutput")
    clsout_d = nc.dram_tensor("clsout", [B, K, RES, RES], f32, kind="ExternalOutput")
    feat_d = nc.dram_tensor("feat_scratch", [B * N, 8], f32, kind="Internal")

    r480 = float(np.float32(1.0) / np.float32(480))
    c075 = 0.75
    bins_np = (np.arange(1, RES, dtype=np.float32) / np.float32(RES))
    ex2 = (np.float32(2.0) * (np.float32(W_CONST / H_CONST) * bins_np)).astype(np.float32)
    ey2 = (np.float32(2.0) * bins_np).astype(np.float32)

    with tile.TileContext(nc) as tc, ExitStack() as ctx:
        const_pool = ctx.enter_context(tc.tile_pool(name="const", bufs=1))
        a_pool = ctx.enter_context(tc.tile_pool(name="stageA", bufs=1))
        rep_pool = ctx.enter_context(tc.tile_pool(name="rep", bufs=1))
        fg_pool = ctx.enter_context(tc.tile_pool(name="fgp", bufs=1))
        work_pool = ctx.enter_context(tc.tile_pool(name="work", bufs=3))
        out_pool = ctx.enter_context(tc.tile_pool(name="outs", bufs=3))

        # ---- constants: per-partition cell id (p % 49) and image offset (p>=49) ----
        pio = const_pool.tile([98, 1], i32)
        nc.gpsimd.iota(pio[:], pattern=[[0, 1]], base=0, channel_multiplier=1)
        piof = const_pool.tile([98, 1], f32)
        nc.vector.tensor_copy(piof[:], pio[:])
        ge49 = const_pool.tile([98, 1], f32)
        nc.vector.tensor_scalar(ge49[:], piof[:], 48.5, scalar2=None, op0=Alu.is_gt)
        g49 = const_pool.tile([98, 1], f32)
        nc.vector.tensor_scalar(g49[:], ge49[:], 49.0, scalar2=None, op0=Alu.mult)
        cvec = const_pool.tile([98, 1], f32)
        nc.vector.tensor_sub(cvec[:], piof[:], g49[:])
        img1000 = const_pool.tile([98, 1], f32)
        nc.vector.tensor_scalar(img1000[:], ge49[:], 1000.0, scalar2=None, op0=Alu.mult)

        # ---- stage A: load + binning + feature table at [128, 250] ----
        s_sb = a_pool.tile([128, 250], f32)
        nc.sync.dma_start(s_sb[:], scores_d[:].rearrange("b (p n) -> (b p) n", p=4))
        b_sb = a_pool.tile([128, 250, 4], f32)
        nc.sync.dma_start(b_sb[:], boxes_d[:].rearrange("b (p n) c -> (b p) n c", p=4))
        l_sb = a_pool.tile([128, 250], f32)
        nc.sync.dma_start(l_sb[:], labels_d[:].rearrange("b (p n) -> (b p) n", p=4))

        nb = a_pool.tile([128, 250, 4], f32)
        nc.vector.tensor_scalar(nb[:], b_sb[:], r480, scalar2=None, op0=Alu.mult)

        sx = a_pool.tile([128, 250], f32)
        nc.vector.tensor_tensor(sx[:], nb[:, :, 0], nb[:, :, 2], op=Alu.add)
        sy = a_pool.tile([128, 250], f32)
        nc.vector.tensor_tensor(sy[:], nb[:, :, 1], nb[:, :, 3], op=Alu.add)

        # px on DVE, py on GPSIMD (parallel engines)
        px = a_pool.tile([128, 250], f32)
        tmp = a_pool.tile([128, 250], f32)
        nc.vector.tensor_scalar(px[:], sx[:], float(ex2[0]), scalar2=None, op0=Alu.is_ge)
        for j in range(1, RES - 1):
            nc.vector.tensor_scalar(tmp[:], sx[:], float(ex2[j]), scalar2=None, op0=Alu.is_ge)
            nc.vector.tensor_add(px[:], px[:], tmp[:])
        py = a_pool.tile([128, 250], f32)
        tmpg = a_pool.tile([128, 250], f32)
        nc.vector.tensor_scalar(py[:], sy[:], float(ey2[0]), scalar2=None, op0=Alu.is_ge)
        for j in range(1, RES - 1):
            nc.vector.tensor_scalar(tmpg[:], sy[:], float(ey2[j]), scalar2=None, op0=Alu.is_ge)
            nc.vector.tensor_add(py[:], py[:], tmpg[:])

        binf = a_pool.tile([128, 250], f32)
        nc.vector.tensor_scalar(binf[:], py[:], 7.0, scalar2=None, op0=Alu.mult)
        nc.vector.tensor_add(binf[:], binf[:], px[:])
        binb = a_pool.tile([128, 250], bf16)
        nc.vector.tensor_copy(binb[:], binf[:])

        keptm = a_pool.tile([128, 250], f32)
        nc.vector.tensor_scalar(keptm[:], s_sb[:], float(np.float32(THRES)), scalar2=None, op0=Alu.is_gt)
        smask = a_pool.tile([128, 250], f32)
        nc.vector.tensor_mul(smask[:], s_sb[:], keptm[:])

        # feature table [128, 250, 8]: score, w, h, x0, y0, label, 0, 0
        ftab = a_pool.tile([128, 250, 8], f32)
        nc.vector.memset(ftab[:, :, 6:8], 0.0)
        nc.vector.tensor_copy(ftab[:, :, 0], s_sb[:])
        nc.vector.tensor_tensor(ftab[:, :, 1], nb[:, :, 2], nb[:, :, 0], op=Alu.subtract)
        nc.vector.tensor_scalar(ftab[:, :, 1], ftab[:, :, 1], c075, scalar2=None, op0=Alu.mult)
        nc.vector.tensor_tensor(ftab[:, :, 2], nb[:, :, 3], nb[:, :, 1], op=Alu.subtract)
        nc.vector.tensor_scalar(ftab[:, :, 3], nb[:, :, 0], c075, scalar2=None, op0=Alu.mult)
        nc.vector.tensor_copy(ftab[:, :, 4], nb[:, :, 1])
        nc.vector.tensor_copy(ftab[:, :, 5], l_sb[:])
        nc.sync.dma_start(feat_d[:].rearrange("(b p n) f -> (b p) n f", p=4, n=250), ftab[:])

        # repack to [32, 1000] rows for replication
        smask32 = a_pool.tile([32, 4, 250], f32)
        nc.sync.dma_start(smask32[:], smask[:])
        binb32 = a_pool.tile([32, 4, 250], bf16)
        nc.sync.dma_start(binb32[:], binb[:])

        # absorb the feat_d HWDGE completion into the Pool queue's clock so the
        # per-tile indirect gathers need only their single DVE wait
        dummy = a_pool.tile([1, 2], f32)
        nc.gpsimd.dma_start(dummy[:], feat_d[0:1, 0:2])

        # ---- stage B/C: 16 tiles of [98, 1000] ----
        for t in range(B // 2):
            smask_rep = rep_pool.tile([98, N], f32, tag=f"smask_rep{t}")
            src_s = (smask32[2 * t : 2 * t + 2, :, :]
                     .rearrange("i p n -> i (p n)")
                     .rearrange("i (o n) -> i o n", o=1)
                     .broadcast_to([2, R2, N]))
            nc.sync.dma_start(smask_rep[:], src_s)
            binb_rep = rep_pool.tile([98, N], bf16, tag=f"binb_rep{t}")
            src_b = (binb32[2 * t : 2 * t + 2, :, :]
                     .rearrange("i p n -> i (p n)")
                     .rearrange("i (o n) -> i o n", o=1)
                     .broadcast_to([2, R2, N]))
            nc.sync.dma_start(binb_rep[:], src_b)

            mask_f = work_pool.tile([98, N], f32, tag="mask_f")
            nc.gpsimd.tensor_scalar(mask_f[:], binb_rep[:], cvec[:], scalar2=None, op0=Alu.is_equal)
            masked = work_pool.tile([98, N], f32, tag="masked")
            nc.vector.tensor_mul(masked[:], smask_rep[:], mask_f[:])

            vals8 = out_pool.tile([98, 8], f32, tag="vals8")
            nc.vector.max(out=vals8[:], in_=masked[:])
            idx8 = out_pool.tile([98, 8], u32, tag="idx8")
            nc.vector.max_index(out=idx8[:], in_max=vals8[:], in_values=masked[:])

            idxf = out_pool.tile([98, K], f32, tag="idxf")
            nc.vector.tensor_copy(idxf[:], idx8[:, 0:K])
            nc.vector.tensor_scalar(idxf[:], idxf[:], float(2 * t * 1000), scalar2=None, op0=Alu.add)
            nc.vector.tensor_tensor(idxf[:], idxf[:], img1000[:].to_broadcast([98, K]), op=Alu.add)
            gidx = out_pool.tile([98, K], i32, tag="gidx")
            nc.vector.tensor_copy(gidx[:], idxf[:])

            fg = fg_pool.tile([98, K, 8], f32, tag=f"fg{t}")
            for k in range(K):
                nc.gpsimd.indirect_dma_start(
                    out=fg[:, k, :], out_offset=None, in_=feat_d[:],
                    in_offset=bass.IndirectOffsetOnAxis(ap=gidx[:, k:k + 1], axis=0),
                )

            valid = out_pool.tile([98, K], f32, tag="valid")
            nc.vector.tensor_scalar(valid[:], vals8[:, 0:K], 0.0, scalar2=None, op0=Alu.is_gt)
            vm1 = out_pool.tile([98, K], f32, tag="vm1")
            nc.vector.tensor_scalar(vm1[:], valid[:], 1.0, scalar2=None, op0=Alu.subtract)

            vfeat = out_pool.tile([98, K, 5], f32, tag="vfeat")
            nc.vector.tensor_tensor(
                vfeat[:], fg[:, :, 0:5],
                valid[:].rearrange("p (k o) -> p k o", o=1).broadcast_to([98, K, 5]),
                op=Alu.mult)
            nc.vector.tensor_tensor(
                vfeat[:], vfeat[:],
                vm1[:].rearrange("p (k o) -> p k o", o=1).broadcast_to([98, K, 5]),
                op=Alu.add)
            vcls = out_pool.tile([98, K], f32, tag="vcls")
            nc.vector.tensor_mul(vcls[:], fg[:, :, 5], valid[:])

            # outputs: boxout[(2t+i), k*5+f, c] <- vfeat[i*49+c, k, f]
            for i in range(2):
                img = 2 * t + i
                dst_box = boxout_d[img].rearrange("(k f) py px -> (py px) k f", k=K)
                nc.sync.dma_start(dst_box, vfeat[i * R2:(i + 1) * R2, :, :])
                dst_cls = clsout_d[img].rearrange("k py px -> (py px) k")
                nc.sync.dma_start(dst_cls, vcls[i * R2:(i + 1) * R2, :])

    if split_waits:
        _split_multi_waits(nc, mybir)
    return nc


def _split_multi_waits(nc, mybir):
    """This walrus build accepts at most one sync-wait per instruction; hoist
    extra waits onto standalone event-semaphore instructions placed directly
    before the offending instruction on the same engine/queue."""
    n_split = 0
    for f in nc.m.functions:
        for bb in f.blocks:
            new_insts = []
            for inst in bb.instructions:
                si = inst.sync_info
                ow = list(si.on_wait) if (si is not None and si.on_wait) else []
                if len(ow) > 1:
                    for k, w in enumerate(ow[:-1]):
                        ev = mybir.InstEventSemaphore(
                            name=f"{inst.name}-wsplit{k}",
                            engine=inst.engine,
                            sync_info=mybir.SyncInfo(on_wait=[w], on_update=[]),
                            bass_nofuse=True,
                        )
                        new_insts.append(ev)
                        n_split += 1
                    si.on_wait = [ow[-1]]
                new_insts.append(inst)
            try:
                bb.instructions[:] = new_insts
            except TypeError:
                bb.instructions = new_insts
    return n_split


def _get_program():
    global _PROGRAM
    if _PROGRAM is None:
        _PROGRAM = _build_program()
    return _PROGRAM


def kernel(boxes, labels, pred_scores, H, W):
    global LAST_RESULTS
    from concourse import bass_utils

    boxes = np.ascontiguousarray(np.asarray(boxes, dtype=np.float32))
    scores = np.ascontiguousarray(np.asarray(pred_scores, dtype=np.float32))
    labels_np = np.asarray(labels)
    labels_f = np.ascontiguousarray(labels_np.astype(np.float32))

    nc = _get_program()
    in_maps = []
    for c in range(N_CORES):
        sl = slice(c * B, (c + 1) * B)
        in_maps.append({
            "scores": scores[sl],
            "boxes": boxes[sl],
            "labels": labels_f[sl],
        })

    res = bass_utils.run_bass_kernel_spmd(nc, in_maps, core_ids=list(range(N_CORES)))
    LAST_RESULTS = res

    cls_parts = []
    box_parts = []
    for c in range(N_CORES):
        out = res.results[c]
        cls_parts.append(out["clsout"])
        box_parts.append(out["boxout"])
    classes = np.concatenate(cls_parts, axis=0)
    boxes_out = np.concatenate(box_parts, axis=0).astype(np.float32)
    classes_i = np.rint(classes).astype(labels_np.dtype if labels_np.dtype.kind == "i" else np.int32)
    return classes_i, boxes_out


# revision 17
# speedup vs baseline: 1.0493x; 1.0493x over previous
"""Trainium2 Bass kernel for nn_BatchedFasterRCNN (histogram binning + per-cell top-3).

Contract: kernel(**inputs) takes FULL inputs (boxes [256,1000,4] f32,
labels [256,1000] int, pred_scores [256,1000] f32, H=480, W=640) and returns
(classes_out [256,3,7,7] int, boxes_out [256,15,7,7] f32), matching reference.

Strategy: pure data-parallel over 8 NeuronCores (32 images each). Per core:
  A) elementwise binning at [128,250] (partition = image-quarter):
     nb = boxes * (1/480); px/py by 6 threshold compares each;
     bin = 7*py+px; smask = score * (score > 0.12); per-box feature table
     [score, w, h, x0, y0, label] staged to DRAM.
  B) 16 tiles of [98,1000] (2 images x 49 cells): replicate smask (f32) and
     bin (bf16) rows across each image's 49 cell-partitions via broadcast-AP
     SBUF->SBUF DMA; mask = (bin == cell); masked = smask * mask;
     vector.max -> top-8 scores; vector.max_index -> their box indices.
  C) indirect-DMA gather of the top-3 boxes' feature rows from the DRAM
     table; validity masking (-1 / 0 fills); DMA to transposed outputs.
"""

import os
import numpy as np

N_CORES = 8
B_FULL, N = 256, 1000
B = B_FULL // N_CORES  # 32 images per core
RES = 7
R2 = RES * RES  # 49
K = 3
THRES = 0.12
H_CONST, W_CONST = 480, 640

LAST_RESULTS = None  # BassKernelResults of the most recent run (for profiling)

_PROGRAM = None


def _build_program(split_waits=True):
    from contextlib import ExitStack

    import concourse.bass as bass
    import concourse.tile as tile
    import concourse.mybir as mybir

    f32 = mybir.dt.float32
    bf16 = mybir.dt.bfloat16
    i32 = mybir.dt.int32
    u32 = mybir.dt.uint32
    Alu = mybir.AluOpType

    nc = bass.Bass()

    scores_d = nc.dram_tensor("scores", [B, N], f32, kind="ExternalInput")
    boxes_d = nc.dram_tensor("boxes", [B, N, 4], f32, kind="ExternalInput")
    labels_d = nc.dram_tensor("labels", [B, N], f32, kind="ExternalInput")
    boxout_d = nc.dram_tensor("boxout", [B, K * 5, RES, RES], f32, kind="ExternalOutput")
    clsout_d = nc.dram_tensor("clsout", [B, K, RES, RES], f32, kind="ExternalOutput")
    feat_d = nc.dram_tensor("feat_scratch", [B * N, 8], f32, kind="Internal")

    r480 = float(np.float32(1.0) / np.float32(480))
    c075 = 0.75
    bins_np = (np.arange(1, RES, dtype=np.float32) / np.float32(RES))
    ex2 = (np.float32(2.0) * (np.float32(W_CONST / H_CONST) * bins_np)).astype(np.float32)
    ey2 = (np.float32(2.0) * bins_np).astype(np.float32)

    with tile.TileContext(nc) as tc, ExitStack() as ctx:
        const_pool = ctx.enter_context(tc.tile_pool(name="const", bufs=1))
        a_pool = ctx.enter_context(tc.tile_pool(name="stageA", bufs=1))
        rep_pool = ctx.enter_context(tc.tile_pool(name="rep", bufs=1))
        fg_pool = ctx.enter_context(tc.tile_pool(name="fgp", bufs=1))
        work_pool = ctx.enter_context(tc.tile_pool(name="work", bufs=3))
        out_pool = ctx.enter_context(tc.tile_pool(name="outs", bufs=3))

        # ---- constants: per-partition cell id (p % 49) and image offset (p>=49) ----
        pio = const_pool.tile([98, 1], i32)
        nc.gpsimd.iota(pio[:], pattern=[[0, 1]], base=0, channel_multiplier=1)
        piof = const_pool.tile([98, 1], f32)
        nc.vector.tensor_copy(piof[:], pio[:])
        ge49 = const_pool.tile([98, 1], f32)
        nc.vector.tensor_scalar(ge49[:], piof[:], 48.5, scalar2=None, op0=Alu.is_gt)
        g49 = const_pool.tile([98, 1], f32)
        nc.vector.tensor_scalar(g49[:], ge49[:], 49.0, scalar2=None, op0=Alu.mult)
        cvec = const_pool.tile([98, 1], f32)
        nc.vector.tensor_sub(cvec[:], piof[:], g49[:])
        img1000 = const_pool.tile([98, 1], f32)
        nc.vector.tensor_scalar(img1000[:], ge49[:], 1000.0, scalar2=None, op0=Alu.mult)

        # ---- stage A: load + binning + feature table at [128, 250] ----
        s_sb = a_pool.tile([128, 250], f32)
        nc.sync.dma_start(s_sb[:], scores_d[:].rearrange("b (p n) -> (b p) n", p=4))
        b_sb = a_pool.tile([128, 250, 4], f32)
        nc.sync.dma_start(b_sb[:], boxes_d[:].rearrange("b (p n) c -> (b p) n c", p=4))
        l_sb = a_pool.tile([128, 250], f32)
        nc.sync.dma_start(l_sb[:], labels_d[:].rearrange("b (p n) -> (b p) n", p=4))

        nb = a_pool.tile([128, 250, 4], f32)
        nc.vector.tensor_scalar(nb[:], b_sb[:], r480, scalar2=None, op0=Alu.mult)

        sx = a_pool.tile([128, 250], f32)
        nc.vector.tensor_tensor(sx[:], nb[:, :, 0], nb[:, :, 2], op=Alu.add)
        sy = a_pool.tile([128, 250], f32)
        nc.vector.tensor_tensor(sy[:], nb[:, :, 1], nb[:, :, 3], op=Alu.add)

        # px on DVE, py on GPSIMD (parallel engines)
        px = a_pool.tile([128, 250], f32)
        tmp = a_pool.tile([128, 250], f32)
        nc.vector.tensor_scalar(px[:], sx[:], float(ex2[0]), scalar2=None, op0=Alu.is_ge)
        for j in range(1, RES - 1):
            nc.vector.tensor_scalar(tmp[:], sx[:], float(ex2[j]), scalar2=None, op0=Alu.is_ge)
            nc.vector.tensor_add(px[:], px[:], tmp[:])
        py = a_pool.tile([128, 250], f32)
        tmpg = a_pool.tile([128, 250], f32)
        nc.vector.tensor_scalar(py[:], sy[:], float(ey2[0]), scalar2=None, op0=Alu.is_ge)
        for j in range(1, RES - 1):
            nc.vector.tensor_scalar(tmpg[:], sy[:], float(ey2[j]), scalar2=None, op0=Alu.is_ge)
            nc.vector.tensor_add(py[:], py[:], tmpg[:])

        binf = a_pool.tile([128, 250], f32)
        nc.vector.tensor_scalar(binf[:], py[:], 7.0, scalar2=None, op0=Alu.mult)
        nc.vector.tensor_add(binf[:], binf[:], px[:])
        binb = a_pool.tile([128, 250], bf16)
        nc.vector.tensor_copy(binb[:], binf[:])

        keptm = a_pool.tile([128, 250], f32)
        nc.vector.tensor_scalar(keptm[:], s_sb[:], float(np.float32(THRES)), scalar2=None, op0=Alu.is_gt)
        smask = a_pool.tile([128, 250], f32)
        nc.vector.tensor_mul(smask[:], s_sb[:], keptm[:])

        # feature table [128, 250, 8]: score, w, h, x0, y0, label, 0, 0
        ftab = a_pool.tile([128, 250, 8], f32)
        nc.vector.memset(ftab[:, :, 6:8], 0.0)
        nc.vector.tensor_copy(ftab[:, :, 0], s_sb[:])
        nc.vector.tensor_tensor(ftab[:, :, 1], nb[:, :, 2], nb[:, :, 0], op=Alu.subtract)
        nc.vector.tensor_scalar(ftab[:, :, 1], ftab[:, :, 1], c075, scalar2=None, op0=Alu.mult)
        nc.vector.tensor_tensor(ftab[:, :, 2], nb[:, :, 3], nb[:, :, 1], op=Alu.subtract)
        nc.vector.tensor_scalar(ftab[:, :, 3], nb[:, :, 0], c075, scalar2=None, op0=Alu.mult)
        nc.vector.tensor_copy(ftab[:, :, 4], nb[:, :, 1])
        nc.vector.tensor_copy(ftab[:, :, 5], l_sb[:])
        nc.sync.dma_start(feat_d[:].rearrange("(b p n) f -> (b p) n f", p=4, n=250), ftab[:])

        # repack to [32, 1000] rows for replication
        smask32 = a_pool.tile([32, 4, 250], f32)
        nc.sync.dma_start(smask32[:], smask[:])
        binb32 = a_pool.tile([32, 4, 250], bf16)
        nc.sync.dma_start(binb32[:], binb[:])

        # absorb the feat_d HWDGE completion into the Pool queue's clock so the
        # per-tile indirect gathers need only their single DVE wait
        dummy = a_pool.tile([1, 2], f32)
        nc.gpsimd.dma_start(dummy[:], feat_d[0:1, 0:2])

        # ---- stage B/C: 16 tiles of [98, 1000] ----
        for t in range(B // 2):
            smask_rep = rep_pool.tile([98, N], f32, tag=f"smask_rep{t}")
            src_s = (smask32[2 * t : 2 * t + 2, :, :]
                     .rearrange("i p n -> i (p n)")
                     .rearrange("i (o n) -> i o n", o=1)
                     .broadcast_to([2, R2, N]))
            nc.sync.dma_start(smask_rep[:], src_s)
            binb_rep = rep_pool.tile([98, N], bf16, tag=f"binb_rep{t}")
            src_b = (binb32[2 * t : 2 * t + 2, :, :]
                     .rearrange("i p n -> i (p n)")
                     .rearrange("i (o n) -> i o n", o=1)
                     .broadcast_to([2, R2, N]))
            nc.sync.dma_start(binb_rep[:], src_b)

            mask_f = work_pool.tile([98, N], f32, tag="mask_f")
            nc.gpsimd.tensor_scalar(mask_f[:], binb_rep[:], cvec[:], scalar2=None, op0=Alu.is_equal)
            masked = work_pool.tile([98, N], f32, tag="masked")
            if t % 4 == 3:
                nc.gpsimd.tensor_mul(masked[:], smask_rep[:], mask_f[:])
            else:
                nc.vector.tensor_mul(masked[:], smask_rep[:], mask_f[:])

            vals8 = out_pool.tile([98, 8], f32, tag="vals8")
            nc.vector.max(out=vals8[:], in_=masked[:])
            idx8 = out_pool.tile([98, 8], u32, tag="idx8")
            nc.vector.max_index(out=idx8[:], in_max=vals8[:], in_values=masked[:])

            idxf = out_pool.tile([98, K], f32, tag="idxf")
            nc.vector.tensor_copy(idxf[:], idx8[:, 0:K])
            nc.vector.tensor_scalar(idxf[:], idxf[:], float(2 * t * 1000), scalar2=None, op0=Alu.add)
            nc.vector.tensor_tensor(idxf[:], idxf[:], img1000[:].to_broadcast([98, K]), op=Alu.add)
            gidx = out_pool.tile([98, K], i32, tag="gidx")
            nc.vector.tensor_copy(gidx[:], idxf[:])

            fg = fg_pool.tile([98, K, 8], f32, tag=f"fg{t}")
            for k in range(K):
                nc.gpsimd.indirect_dma_start(
                    out=fg[:, k, :], out_offset=None, in_=feat_d[:],
                    in_offset=bass.IndirectOffsetOnAxis(ap=gidx[:, k:k + 1], axis=0),
                )

            valid = out_pool.tile([98, K], f32, tag="valid")
            nc.vector.tensor_scalar(valid[:], vals8[:, 0:K], 0.0, scalar2=None, op0=Alu.is_gt)
            vm1 = out_pool.tile([98, K], f32, tag="vm1")
            nc.vector.tensor_scalar(vm1[:], valid[:], 1.0, scalar2=None, op0=Alu.subtract)

            vfeat = out_pool.tile([98, K, 5], f32, tag="vfeat")
            nc.vector.tensor_tensor(
                vfeat[:], fg[:, :, 0:5],
                valid[:].rearrange("p (k o) -> p k o", o=1).broadcast_to([98, K, 5]),
                op=Alu.mult)
            nc.vector.tensor_tensor(
                vfeat[:], vfeat[:],
                vm1[:].rearrange("p (k o) -> p k o", o=1).broadcast_to([98, K, 5]),
                op=Alu.add)
            vcls = out_pool.tile([98, K], f32, tag="vcls")
            nc.vector.tensor_mul(vcls[:], fg[:, :, 5], valid[:])

            # outputs: boxout[(2t+i), k*5+f, c] <- vfeat[i*49+c, k, f]
            for i in range(2):
                img = 2 * t + i
                dst_box = boxout_d[img].rearrange("(k f) py px -> (py px) k f", k=K)
                nc.sync.dma_start(dst_box, vfeat[i * R2:(i + 1) * R2, :, :])
                dst_cls = clsout_d[img].rearrange("k py px -> (py px) k")
                nc.sync.dma_start(dst_cls, vcls[i * R2:(i + 1) * R2, :])

    if split_waits:
        _split_multi_waits(nc, mybir)
    return nc


def _split_multi_waits(nc, mybir):
    """This walrus build accepts at most one sync-wait per instruction; hoist
    extra waits onto standalone event-semaphore instructions placed directly
    before the offending instruction on the same engine/queue."""
    n_split = 0
    for f in nc.m.functions:
        for bb in f.blocks:
            new_insts = []
            for inst in bb.instructions:
                si = inst.sync_info
                ow = list(si.on_wait) if (si is not None and si.on_wait) else []
                if len(ow) > 1:
                    for k, w in enumerate(ow[:-1]):
                        ev = mybir.InstEventSemaphore(
                            name=f"{inst.name}-wsplit{k}",
                            engine=inst.engine,
                            sync_info=mybir.SyncInfo(on_wait=[w], on_update=[]),
                            bass_nofuse=True,
                        )
                        new_insts.append(ev)
                        n_split += 1
                    si.on_wait = [ow[-1]]
                new_insts.append(inst)
            try:
                bb.instructions[:] = new_insts
            except TypeError:
                bb.instructions = new_insts
    return n_split


def _get_program():
    global _PROGRAM
    if _PROGRAM is None:
        _PROGRAM = _build_program()
    return _PROGRAM


def kernel(boxes, labels, pred_scores, H, W):
    global LAST_RESULTS
    from concourse import bass_utils

    boxes = np.ascontiguousarray(np.asarray(boxes, dtype=np.float32))
    scores = np.ascontiguousarray(np.asarray(pred_scores, dtype=np.float32))
    labels_np = np.asarray(labels)
    labels_f = np.ascontiguousarray(labels_np.astype(np.float32))

    nc = _get_program()
    in_maps = []
    for c in range(N_CORES):
        sl = slice(c * B, (c + 1) * B)
        in_maps.append({
            "scores": scores[sl],
            "boxes": boxes[sl],
            "labels": labels_f[sl],
        })

    res = bass_utils.run_bass_kernel_spmd(nc, in_maps, core_ids=list(range(N_CORES)))
    LAST_RESULTS = res

    cls_parts = []
    box_parts = []
    for c in range(N_CORES):
        out = res.results[c]
        cls_parts.append(out["clsout"])
        box_parts.append(out["boxout"])
    classes = np.concatenate(cls_parts, axis=0)
    boxes_out = np.concatenate(box_parts, axis=0).astype(np.float32)
    classes_i = np.rint(classes).astype(labels_np.dtype if labels_np.dtype.kind == "i" else np.int32)
    return classes_i, boxes_out


# revision 20
# speedup vs baseline: 1.1059x; 1.0539x over previous
"""Trainium2 Bass kernel for nn_BatchedFasterRCNN (histogram binning + per-cell top-3).

Contract: kernel(**inputs) takes FULL inputs (boxes [256,1000,4] f32,
labels [256,1000] int, pred_scores [256,1000] f32, H=480, W=640) and returns
(classes_out [256,3,7,7] int, boxes_out [256,15,7,7] f32), matching reference.

Strategy: pure data-parallel over 8 NeuronCores (32 images each). Per core:
  A) elementwise binning at [128,250] (partition = image-quarter):
     nb = boxes * (1/480); px/py by 6 threshold compares each;
     bin = 7*py+px; smask = score * (score > 0.12); per-box feature table
     [score, w, h, x0, y0, label] staged to DRAM.
  B) 16 tiles of [98,1000] (2 images x 49 cells): replicate smask (f32) and
     bin (bf16) rows across each image's 49 cell-partitions via broadcast-AP
     SBUF->SBUF DMA; mask = (bin == cell); masked = smask * mask;
     vector.max -> top-8 scores; vector.max_index -> their box indices.
  C) indirect-DMA gather of the top-3 boxes' feature rows from the DRAM
     table; validity masking (-1 / 0 fills); DMA to transposed outputs.
"""

import os
import numpy as np

N_CORES = 8
B_FULL, N = 256, 1000
B = B_FULL // N_CORES  # 32 images per core
RES = 7
R2 = RES * RES  # 49
K = 3
THRES = 0.12
H_CONST, W_CONST = 480, 640

LAST_RESULTS = None  # BassKernelResults of the most recent run (for profiling)

_PROGRAM = None


def _build_program(split_waits=True):
    from contextlib import ExitStack

    import concourse.bass as bass
    import concourse.tile as tile
    import concourse.mybir as mybir

    f32 = mybir.dt.float32
    bf16 = mybir.dt.bfloat16
    i32 = mybir.dt.int32
    u32 = mybir.dt.uint32
    Alu = mybir.AluOpType

    nc = bass.Bass()

    scores_d = nc.dram_tensor("scores", [B, N], f32, kind="ExternalInput")
    boxes_d = nc.dram_tensor("boxes", [B, N, 4], f32, kind="ExternalInput")
    labels_d = nc.dram_tensor("labels", [B, N], f32, kind="ExternalInput")
    boxout_d = nc.dram_tensor("boxout", [B, K * 5, RES, RES], f32, kind="ExternalOutput")
    clsout_d = nc.dram_tensor("clsout", [B, K, RES, RES], f32, kind="ExternalOutput")
    feat_d = nc.dram_tensor("feat_scratch", [B * N, 8], f32, kind="Internal")

    r480 = float(np.float32(1.0) / np.float32(480))
    c075 = 0.75
    bins_np = (np.arange(1, RES, dtype=np.float32) / np.float32(RES))
    ex2 = (np.float32(2.0) * (np.float32(W_CONST / H_CONST) * bins_np)).astype(np.float32)
    ey2 = (np.float32(2.0) * bins_np).astype(np.float32)

    with tile.TileContext(nc) as tc, ExitStack() as ctx:
        const_pool = ctx.enter_context(tc.tile_pool(name="const", bufs=1))
        a_pool = ctx.enter_context(tc.tile_pool(name="stageA", bufs=1))
        rep_pool = ctx.enter_context(tc.tile_pool(name="rep", bufs=1))
        fg_pool = ctx.enter_context(tc.tile_pool(name="fgp", bufs=1))
        work_pool = ctx.enter_context(tc.tile_pool(name="work", bufs=4))
        out_pool = ctx.enter_context(tc.tile_pool(name="outs", bufs=3))

        # ---- constants: per-partition cell id (p % 49) and image offset (p>=49) ----
        pio = const_pool.tile([98, 1], i32)
        nc.gpsimd.iota(pio[:], pattern=[[0, 1]], base=0, channel_multiplier=1)
        piof = const_pool.tile([98, 1], f32)
        nc.vector.tensor_copy(piof[:], pio[:])
        ge49 = const_pool.tile([98, 1], f32)
        nc.vector.tensor_scalar(ge49[:], piof[:], 48.5, scalar2=None, op0=Alu.is_gt)
        g49 = const_pool.tile([98, 1], f32)
        nc.vector.tensor_scalar(g49[:], ge49[:], 49.0, scalar2=None, op0=Alu.mult)
        cvec = const_pool.tile([98, 1], f32)
        nc.vector.tensor_sub(cvec[:], piof[:], g49[:])
        img1000 = const_pool.tile([98, 1], f32)
        nc.vector.tensor_scalar(img1000[:], ge49[:], 1000.0, scalar2=None, op0=Alu.mult)

        # ---- stage A: load + binning + feature table at [128, 250] ----
        s_sb = a_pool.tile([128, 250], f32)
        nc.sync.dma_start(s_sb[:], scores_d[:].rearrange("b (p n) -> (b p) n", p=4))
        b_sb = a_pool.tile([128, 250, 4], f32)
        nc.sync.dma_start(b_sb[:], boxes_d[:].rearrange("b (p n) c -> (b p) n c", p=4))
        l_sb = a_pool.tile([128, 250], f32)
        nc.sync.dma_start(l_sb[:], labels_d[:].rearrange("b (p n) -> (b p) n", p=4))

        nb = a_pool.tile([128, 250, 4], f32)
        nc.vector.tensor_scalar(nb[:], b_sb[:], r480, scalar2=None, op0=Alu.mult)

        sx = a_pool.tile([128, 250], f32)
        nc.vector.tensor_tensor(sx[:], nb[:, :, 0], nb[:, :, 2], op=Alu.add)
        sy = a_pool.tile([128, 250], f32)
        nc.gpsimd.tensor_tensor(sy[:], nb[:, :, 1], nb[:, :, 3], op=Alu.add)

        # px on DVE, py on GPSIMD (parallel engines)
        px = a_pool.tile([128, 250], f32)
        tmp = a_pool.tile([128, 250], f32)
        nc.vector.tensor_scalar(px[:], sx[:], float(ex2[0]), scalar2=None, op0=Alu.is_ge)
        for j in range(1, RES - 1):
            nc.vector.tensor_scalar(tmp[:], sx[:], float(ex2[j]), scalar2=None, op0=Alu.is_ge)
            nc.vector.tensor_add(px[:], px[:], tmp[:])
        py = a_pool.tile([128, 250], f32)
        tmpg = a_pool.tile([128, 250], f32)
        nc.gpsimd.tensor_scalar(py[:], sy[:], float(ey2[0]), scalar2=None, op0=Alu.is_ge)
        for j in range(1, RES - 1):
            nc.gpsimd.tensor_scalar(tmpg[:], sy[:], float(ey2[j]), scalar2=None, op0=Alu.is_ge)
            nc.gpsimd.tensor_add(py[:], py[:], tmpg[:])

        binf = a_pool.tile([128, 250], f32)
        nc.vector.tensor_scalar(binf[:], py[:], 7.0, scalar2=None, op0=Alu.mult)
        nc.vector.tensor_add(binf[:], binf[:], px[:])
        binb = a_pool.tile([128, 250], bf16)
        nc.vector.tensor_copy(binb[:], binf[:])

        keptm = a_pool.tile([128, 250], f32)
        nc.vector.tensor_scalar(keptm[:], s_sb[:], float(np.float32(THRES)), scalar2=None, op0=Alu.is_gt)
        smask = a_pool.tile([128, 250], f32)
        nc.vector.tensor_mul(smask[:], s_sb[:], keptm[:])

        # feature table [128, 250, 8]: score, w, h, x0, y0, label, 0, 0
        ftab = a_pool.tile([128, 250, 8], f32)
        nc.vector.memset(ftab[:, :, 6:8], 0.0)
        nc.vector.tensor_copy(ftab[:, :, 0], s_sb[:])
        nc.vector.tensor_tensor(ftab[:, :, 1], nb[:, :, 2], nb[:, :, 0], op=Alu.subtract)
        nc.vector.tensor_scalar(ftab[:, :, 1], ftab[:, :, 1], c075, scalar2=None, op0=Alu.mult)
        nc.vector.tensor_tensor(ftab[:, :, 2], nb[:, :, 3], nb[:, :, 1], op=Alu.subtract)
        nc.vector.tensor_scalar(ftab[:, :, 3], nb[:, :, 0], c075, scalar2=None, op0=Alu.mult)
        nc.vector.tensor_copy(ftab[:, :, 4], nb[:, :, 1])
        nc.vector.tensor_copy(ftab[:, :, 5], l_sb[:])
        nc.sync.dma_start(feat_d[:].rearrange("(b p n) f -> (b p) n f", p=4, n=250), ftab[:])

        # repack to [32, 1000] rows for replication
        smask32 = a_pool.tile([32, 4, 250], f32)
        nc.sync.dma_start(smask32[:], smask[:])
        binb32 = a_pool.tile([32, 4, 250], bf16)
        nc.sync.dma_start(binb32[:], binb[:])

        # absorb the feat_d HWDGE completion into the Pool queue's clock so the
        # per-tile indirect gathers need only their single DVE wait
        dummy = a_pool.tile([1, 2], f32)
        nc.gpsimd.dma_start(dummy[:], feat_d[0:1, 0:2])

        # ---- stage B/C: 16 tiles of [98, 1000] ----
        for t in range(B // 2):
            smask_rep = rep_pool.tile([98, N], f32, tag=f"smask_rep{t}")
            src_s = (smask32[2 * t : 2 * t + 2, :, :]
                     .rearrange("i p n -> i (p n)")
                     .rearrange("i (o n) -> i o n", o=1)
                     .broadcast_to([2, R2, N]))
            nc.sync.dma_start(smask_rep[:], src_s)
            binb_rep = rep_pool.tile([98, N], bf16, tag=f"binb_rep{t}")
            src_b = (binb32[2 * t : 2 * t + 2, :, :]
                     .rearrange("i p n -> i (p n)")
                     .rearrange("i (o n) -> i o n", o=1)
                     .broadcast_to([2, R2, N]))
            nc.sync.dma_start(binb_rep[:], src_b)

            mask_f = work_pool.tile([98, N], f32, tag="mask_f")
            nc.gpsimd.tensor_scalar(mask_f[:], binb_rep[:], cvec[:], scalar2=None, op0=Alu.is_equal)
            masked = work_pool.tile([98, N], f32, tag="masked")
            if t % 2 == 1:
                nc.gpsimd.tensor_mul(masked[:], smask_rep[:], mask_f[:])
            else:
                nc.vector.tensor_mul(masked[:], smask_rep[:], mask_f[:])

            vals8 = out_pool.tile([98, 8], f32, tag="vals8")
            nc.vector.max(out=vals8[:], in_=masked[:])
            idx8 = out_pool.tile([98, 8], u32, tag="idx8")
            nc.vector.max_index(out=idx8[:], in_max=vals8[:], in_values=masked[:])

            idxf = out_pool.tile([98, K], f32, tag="idxf")
            nc.vector.tensor_copy(idxf[:], idx8[:, 0:K])
            nc.vector.tensor_scalar(idxf[:], idxf[:], float(2 * t * 1000), scalar2=None, op0=Alu.add)
            nc.vector.tensor_tensor(idxf[:], idxf[:], img1000[:].to_broadcast([98, K]), op=Alu.add)
            gidx = out_pool.tile([98, K], i32, tag="gidx")
            nc.vector.tensor_copy(gidx[:], idxf[:])

            fg = fg_pool.tile([98, K, 8], f32, tag=f"fg{t}")
            for k in range(K):
                nc.gpsimd.indirect_dma_start(
                    out=fg[:, k, :], out_offset=None, in_=feat_d[:],
                    in_offset=bass.IndirectOffsetOnAxis(ap=gidx[:, k:k + 1], axis=0),
                )

            valid = out_pool.tile([98, K], f32, tag="valid")
            nc.vector.tensor_scalar(valid[:], vals8[:, 0:K], 0.0, scalar2=None, op0=Alu.is_gt)
            vm1 = out_pool.tile([98, K], f32, tag="vm1")
            nc.vector.tensor_scalar(vm1[:], valid[:], 1.0, scalar2=None, op0=Alu.subtract)

            vfeat = out_pool.tile([98, K, 5], f32, tag="vfeat")
            nc.vector.tensor_tensor(
                vfeat[:], fg[:, :, 0:5],
                valid[:].rearrange("p (k o) -> p k o", o=1).broadcast_to([98, K, 5]),
                op=Alu.mult)
            nc.vector.tensor_tensor(
                vfeat[:], vfeat[:],
                vm1[:].rearrange("p (k o) -> p k o", o=1).broadcast_to([98, K, 5]),
                op=Alu.add)
            vcls = out_pool.tile([98, K], f32, tag="vcls")
            nc.vector.tensor_mul(vcls[:], fg[:, :, 5], valid[:])

            # outputs: boxout[(2t+i), k*5+f, c] <- vfeat[i*49+c, k, f]
            for i in range(2):
                img = 2 * t + i
                dst_box = boxout_d[img].rearrange("(k f) py px -> (py px) k f", k=K)
                nc.sync.dma_start(dst_box, vfeat[i * R2:(i + 1) * R2, :, :])
                dst_cls = clsout_d[img].rearrange("k py px -> (py px) k")
                nc.sync.dma_start(dst_cls, vcls[i * R2:(i + 1) * R2, :])

    if split_waits:
        _split_multi_waits(nc, mybir)
    return nc


def _split_multi_waits(nc, mybir):
    """This walrus build accepts at most one sync-wait per instruction; hoist
    extra waits onto standalone event-semaphore instructions placed directly
    before the offending instruction on the same engine/queue."""
    n_split = 0
    for f in nc.m.functions:
        for bb in f.blocks:
            new_insts = []
            for inst in bb.instructions:
                si = inst.sync_info
                ow = list(si.on_wait) if (si is not None and si.on_wait) else []
                if len(ow) > 1:
                    for k, w in enumerate(ow[:-1]):
                        ev = mybir.InstEventSemaphore(
                            name=f"{inst.name}-wsplit{k}",
                            engine=inst.engine,
                            sync_info=mybir.SyncInfo(on_wait=[w], on_update=[]),
                            bass_nofuse=True,
                        )
                        new_insts.append(ev)
                        n_split += 1
                    si.on_wait = [ow[-1]]
                new_insts.append(inst)
            try:
                bb.instructions[:] = new_insts
            except TypeError:
                bb.instructions = new_insts
    return n_split


def _get_program():
    global _PROGRAM
    if _PROGRAM is None:
        _PROGRAM = _build_program()
    return _PROGRAM


def kernel(boxes, labels, pred_scores, H, W):
    global LAST_RESULTS
    from concourse import bass_utils

    boxes = np.ascontiguousarray(np.asarray(boxes, dtype=np.float32))
    scores = np.ascontiguousarray(np.asarray(pred_scores, dtype=np.float32))
    labels_np = np.asarray(labels)
    labels_f = np.ascontiguousarray(labels_np.astype(np.float32))

    nc = _get_program()
    in_maps = []
    for c in range(N_CORES):
        sl = slice(c * B, (c + 1) * B)
        in_maps.append({
            "scores": scores[sl],
            "boxes": boxes[sl],
            "labels": labels_f[sl],
        })

    res = bass_utils.run_bass_kernel_spmd(nc, in_maps, core_ids=list(range(N_CORES)))
    LAST_RESULTS = res

    cls_parts = []
    box_parts = []
    for c in range(N_CORES):
        out = res.results[c]
        cls_parts.append(out["clsout"])
        box_parts.append(out["boxout"])
    classes = np.concatenate(cls_parts, axis=0)
    boxes_out = np.concatenate(box_parts, axis=0).astype(np.float32)
    classes_i = np.rint(classes).astype(labels_np.dtype if labels_np.dtype.kind == "i" else np.int32)
    return classes_i, boxes_out
